# revision 9
# baseline (speedup 1.0000x reference)
"""Causal single-head attention (B=4, S=2048, d=1024, f32) on 8 TRN2 NeuronCores.

v5: v4's fully factorized, collective-free form (scores = x (Wq Wk^T) x^T via
host-precomputed M; out = (P x) Wv) with a finer causal decomposition:
128-row query blocks instead of 256.

The 16 query blocks of a batch are paired (2t, 2t+1) into 8 slots; the two
cores of a batch take alternating members (h=0: blocks 0,3,4,7,8,11,12,15;
h=1: 1,2,5,6,9,10,13,14) so every core runs the same uniform program with
slot t covering nk_t = 2t+2 key tiles. Total causal tiles per core drop from
40 (256-blocks) to 72 half-width (= 36 equivalent), cutting ~16k PE cycles.
Only the last TWO key tiles of each slot intersect the causal boundary, so the
host mask for slot t is just [256, 128] — 512KB of masks instead of 2.4MB.

Everything else matches v4: Q'^T = M^T xq^T computed qf-outer; scores
stationary = the resident x^T tile; U^T = x-contracted P^T per slot; out =
U Wv with full Wv in SBUF; ones-column denominator matmul folded into the
PSUM->SBUF copy; bf16 compute with f32 PSUM; DMA blessing via DVE copies.
"""

import numpy as np
import ml_dtypes

import concourse.bass as bass
from concourse import bacc
import concourse.mybir as mybir
from concourse.tile import TileContext
from concourse.bass_utils import run_bass_kernel_spmd

P = 128
B = 4
S = 2048          # sequence length (= keys per batch)
D = 1024          # d_in = d_out
HALF = 1024       # queries per core
NSLOT = 8         # query slots per core, 128 rows each
CD = D // P       # 8 contraction tiles
SK = S // P       # 16 key tiles
F = 512           # matmul moving free dim (one PSUM bank of f32)
SCALE = 1.0 / 32.0    # 1/sqrt(d_k)
NEG = -1.0e30         # additive mask for disallowed (k, q)

# slot t has nk_t = 2t+2 key tiles; the last two straddle the causal boundary
NK = tuple(2 * t + 2 for t in range(NSLOT))

# global query-row start per (half, slot): h=0 takes block 2t+(t%2),
# h=1 takes block 2t+1-(t%2); both cores' slot-t prefix maxes at (2t+2)*128
QROWS = (
    tuple((2 * t + (t % 2)) * P for t in range(NSLOT)),
    tuple((2 * t + 1 - (t % 2)) * P for t in range(NSLOT)),
)

BF16 = ml_dtypes.bfloat16


def build_nc(reps: int = 1) -> bacc.Bacc:
    nc = bacc.Bacc("TRN2")
    bf = mybir.dt.bfloat16
    f32 = mybir.dt.float32

    xkv_d = nc.declare_dram_parameter("xkv", [D, S], bf, isOutput=False)
    xun_d = nc.declare_dram_parameter("xun", [S, D], bf, isOutput=False)
    xq_d = nc.declare_dram_parameter("xq", [D, HALF], bf, isOutput=False)
    wqk_d = nc.declare_dram_parameter("wqk", [D, D], bf, isOutput=False)
    wv_d = nc.declare_dram_parameter("wv", [D, D], bf, isOutput=False)
    m_d = [
        nc.declare_dram_parameter(f"mask{t}", [2 * P, P], bf, isOutput=False)
        for t in range(NSLOT)
    ]
    out_d = nc.declare_dram_parameter("out", [HALF, D], bf, isOutput=True)

    with TileContext(nc) as tc:
        with tc.tile_pool(name="persist", bufs=1) as persist, \
             tc.tile_pool(name="work", bufs=1) as work, \
             tc.tile_pool(name="psum", bufs=1, space="PSUM") as psum:
            QT = persist.tile([P, CD, HALF], bf)
            ones = persist.tile([P, 1], bf)
            nc.vector.memset(ones[:], 1.0)

            def load(dst, dram_t, c):
                nc.sync.dma_start(out=dst[:, c], in_=dram_t[c * P:(c + 1) * P, :])
                nc.vector.tensor_copy(dst[:, c], dst[:, c])

            for _rep in range(reps):
                xq_s = work.tile([P, CD, HALF], bf, tag="xq")
                wqk_s = work.tile([P, CD, D], bf, tag="wqk")
                xkv_s = work.tile([P, CD, S], bf, tag="big")
                xun_s = work.tile([P, SK, D], bf, tag="xun")
                wv_s = work.tile([P, CD, D], bf, tag="wv", bufs=2)
                for c in range(CD):
                    load(xq_s, xq_d, c)
                    load(wqk_s, wqk_d, c)
                for c in range(CD):
                    load(xkv_s, xkv_d, c)
                    load(wv_s, wv_d, c)
                for c in range(SK):
                    load(xun_s, xun_d, c)

                # ---------------- Q'^T = M^T xq^T (qf-outer) ----------------
                for qf in range(HALF // F):
                    for m in range(CD):
                        ps = psum.tile([P, F], f32, tag="pp", bufs=3)
                        for c in range(CD):
                            nc.tensor.matmul(
                                ps[:],
                                wqk_s[:, c, m * P:(m + 1) * P],
                                xq_s[:, c, qf * F:(qf + 1) * F],
                                start=(c == 0), stop=(c == CD - 1),
                            )
                        nc.vector.tensor_copy(QT[:, m, qf * F:(qf + 1) * F], ps[:])

                # ---------------- attention (8 slots of 128 queries) --------
                for t in range(NSLOT):
                    nk = NK[t]
                    qb = t * P
                    md = m_d[t]
                    PT = work.tile([P, SK, P], bf, tag="pt")
                    # scores^T[k, q] = sum_c x^T[c, k]^T Q'^T[c, q]; only the
                    # last two key tiles straddle the causal boundary
                    for ki in range(nk):
                        masked = ki >= nk - 2
                        if masked:
                            mt = work.tile([P, P], bf, tag="mask", bufs=4)
                            mrow = (ki - (nk - 2)) * P
                            nc.sync.dma_start(
                                out=mt[:], in_=md[mrow:mrow + P, :])
                            nc.vector.tensor_copy(mt[:], mt[:])
                        ps = psum.tile([P, P], f32, tag="pp", bufs=3)
                        for c in range(CD):
                            nc.tensor.matmul(
                                ps[:],
                                xkv_s[:, c, ki * P:(ki + 1) * P],
                                QT[:, c, qb:qb + P],
                                start=(c == 0), stop=(c == CD - 1),
                            )
                        if masked:
                            nc.vector.tensor_add(ps[:], ps[:], mt[:])
                        pe = work.tile([P, P], bf, tag="pexp", bufs=2)
                        nc.scalar.activation(
                            pe[:], ps[:],
                            mybir.ActivationFunctionType.Exp, scale=SCALE,
                        )
                        nc.vector.tensor_copy(PT[:, ki], pe[:])
                    # U^T[d, q] = sum_k x[k, d]^T P^T[k, q]
                    UT = work.tile([P, CD, P], bf, tag="ut", bufs=2)
                    for db in range(CD):
                        pu = psum.tile([P, P], f32, tag="pp", bufs=3)
                        for ki in range(nk):
                            nc.tensor.matmul(
                                pu[:],
                                xun_s[:, ki, db * P:(db + 1) * P],
                                PT[:, ki, 0:P],
                                start=(ki == 0), stop=(ki == nk - 1),
                            )
                        nc.vector.tensor_copy(UT[:, db, :], pu[:])
                    # out[q, n] = sum_d U^T[d, q]^T Wv[d, n] + denominator
                    o0 = psum.tile([P, F], f32, tag="av", bufs=3)
                    o1 = psum.tile([P, F], f32, tag="av", bufs=3)
                    rs = psum.tile([P, 1], f32, tag="rs", bufs=2)
                    for ki in range(nk):
                        nc.tensor.matmul(rs[:], PT[:, ki, :], ones[:, 0:1],
                                         start=(ki == 0), stop=(ki == nk - 1))
                    for c in range(CD):
                        st_, sp_ = (c == 0), (c == CD - 1)
                        lh = UT[:, c, :]
                        nc.tensor.matmul(o0[:], lh, wv_s[:, c, 0:F],
                                         start=st_, stop=sp_)
                        nc.tensor.matmul(o1[:], lh, wv_s[:, c, F:2 * F],
                                         start=st_, stop=sp_)
                    rcp = work.tile([P, 1], f32, tag="rcp", bufs=4)
                    nc.vector.reciprocal(rcp[:], rs[:])
                    ot = work.tile([P, D], bf, tag="ot", bufs=4)
                    nc.vector.tensor_scalar_mul(ot[:, 0:F], o0[:], rcp[:])
                    nc.vector.tensor_scalar_mul(ot[:, F:2 * F], o1[:], rcp[:])
                    nc.sync.dma_start(out=out_d[qb:qb + P, :], in_=ot[:])
    nc.finalize()
    return nc


_NC_CACHE = {}


def _get_nc(reps: int = 1):
    if reps not in _NC_CACHE:
        _NC_CACHE[reps] = build_nc(reps)
    return _NC_CACHE[reps]


def _masks():
    """Per (half, slot): additive bf16 mask [256, 128] for the last two key
    tiles (global keys 256t..256t+255) vs the slot's 128 queries."""
    q = np.arange(P)[None, :]
    out = []
    for h in range(2):
        ms = []
        for t in range(NSLOT):
            kr = np.arange(2 * P)[:, None] + (NK[t] - 2) * P
            ms.append(np.where(kr <= q + QROWS[h][t], 0.0, NEG).astype(BF16))
        out.append(ms)
    return out


def make_in_maps(x, Wq, Wk, Wv):
    M = np.asarray(Wq, np.float32) @ np.asarray(Wk, np.float32).T
    wqkb = np.ascontiguousarray(M.astype(BF16))
    wvb = np.ascontiguousarray(np.asarray(Wv).astype(BF16))
    masks = _masks()
    in_maps = []
    for i in range(8):
        b, h = i // 2, i % 2
        xb = x[b].astype(BF16)
        xT = np.ascontiguousarray(xb.T)
        xq = np.concatenate([xb[r:r + P] for r in QROWS[h]], axis=0)
        xqT = np.ascontiguousarray(xq.T)
        m = {
            "xkv": xT, "xun": np.ascontiguousarray(xb), "xq": xqT,
            "wqk": wqkb, "wv": wvb,
        }
        for t in range(NSLOT):
            m[f"mask{t}"] = masks[h][t]
        in_maps.append(m)
    return in_maps


def gather_out(results, x_dtype=np.float32):
    out = np.empty((B, S, D), x_dtype)
    for i in range(8):
        b, h = i // 2, i % 2
        o = np.asarray(results[i]["out"]).astype(x_dtype)
        for t, r in enumerate(QROWS[h]):
            out[b, r:r + P] = o[t * P:(t + 1) * P]
    return out


def run_cores(in_maps, **kwargs):
    return run_bass_kernel_spmd(_get_nc(), in_maps, core_ids=list(range(8)), **kwargs)


def kernel(x, Wq, Wk, Wv):
    x = np.asarray(x)
    in_maps = make_in_maps(x, np.asarray(Wq), np.asarray(Wk), np.asarray(Wv))
    res = run_cores(in_maps)
    return gather_out(res.results)
